# revision 1
# baseline (speedup 1.0000x reference)
"""Trainium2 Bass kernel for nn_ChannelizedLinearCompression.

Computation (fp32 reference):
    h1      = relu(einsum('bcn,cnh->bch', x, W1) + b1)   # [B, C, H]
    h2      = relu(einsum('bch,chk->bck', h1, W2) + b2)  # [B, C, 10]
    scalars = einsum('bck,ck->bc', h2, W3) + b3          # [B, C]
    out     = relu(scalars @ Wf1 + bf1) @ Wf2 + bf2      # [B, 16]

Sharding: 2 batch groups x 4 channel groups over 8 cores. Each core gets
x^T[c_loc, N, b_loc] (host-transposed so every big DMA is contiguous) and
computes scalars^T[c_loc, b_loc] on device; the tiny final MLP (0.003% of
the FLOPs) runs on host.

Device per-core dataflow (per local channel c):
  stage1: for each K chunk (128 of N=8192): psum[h_chunk][b_half] +=
          W1[k,h_chunk].T @ xT[k, b_half]   (M=h orientation: h1 lands
          h-major so stages 2/3 chain with no transposes, and b1 bias is
          a per-partition ScalarE activation bias)
  stage2: psum2[b_half] += W2[h_chunk].T @ h1T[h_chunk, b_half]; relu+b2
  stage3: psum3[b_half] = W3.T @ h2T[:, b_half]; +b3 -> scalars^T row
"""

import os
from contextlib import ExitStack

import numpy as np

import concourse.bass as bass
import concourse.tile as tile
from concourse import bacc, mybir
from concourse.bass_utils import run_bass_kernel_spmd
from concourse._compat import get_trn_type

# Problem shapes (hardcoded; kernel.py must be self-contained).
B, C, N = 2048, 12, 8192
H, MID = 286, 10
FINAL_HIDDEN, LOWDIM = 30, 16
BG, CG = 2, 4  # batch groups x channel groups = 8 cores
B_LOC, C_LOC = B // BG, C // CG

_DT_NAMES = {
    "float32r": mybir.dt.float32r,
    "float32": mybir.dt.float32,
    "bfloat16": mybir.dt.bfloat16,
    "float16": mybir.dt.float16,
}
# Stage-1 (big GEMM) operand dtype. float16 halves the HBM traffic (the
# bottleneck), streams the PE at full rate, and keeps ~11 mantissa bits —
# x values (|x|<6) and W1 (~0.02) sit comfortably in fp16 range.
DT1 = _DT_NAMES[os.environ.get("KERNEL_DT", "float16")]

F32 = mybir.dt.float32
RELU = mybir.ActivationFunctionType.Relu
IDENT = mybir.ActivationFunctionType.Identity

LAST = {}  # introspection for test.py (exec_time_ns etc.); harness ignores


def build_nc(b_loc=B_LOC, c_loc=C_LOC, n=N, dt1=DT1):
    assert n % 128 == 0 and b_loc % 512 == 0
    nk = n // 128
    nj = b_loc // 512
    hch = [(i * 128, min(128, H - i * 128)) for i in range((H + 127) // 128)]

    # float32r operands must be produced by a rounding compute op (BIR
    # verifier rejects DMA-fed fp32r matmuls), so DRAM I/O stays float32 and
    # a DVE tensor_copy rounds each tile to fp32r on-chip.
    round_fp32r = dt1 == mybir.dt.float32r
    io_dt1 = F32 if round_fp32r else dt1
    # stages 2/3 are tiny; run them in the 2-byte dtype when stage 1 uses one
    dt2 = dt1 if dt1 in (mybir.dt.float16, mybir.dt.bfloat16) else F32

    nc = bacc.Bacc(get_trn_type() or "TRN2", target_bir_lowering=False)
    xt = nc.declare_dram_parameter("xt", [c_loc, n, b_loc], io_dt1,
                                   isOutput=False)
    w1 = nc.declare_dram_parameter("w1", [c_loc, n, H], io_dt1, isOutput=False)
    b1 = nc.declare_dram_parameter("b1", [c_loc, H, 1], F32, isOutput=False)
    w2 = nc.declare_dram_parameter("w2", [c_loc, H, MID], dt2, isOutput=False)
    b2 = nc.declare_dram_parameter("b2", [c_loc, MID, 1], F32, isOutput=False)
    w3 = nc.declare_dram_parameter("w3", [c_loc, MID, 1], dt2, isOutput=False)
    b3 = nc.declare_dram_parameter("b3", [c_loc, 1, 1], F32, isOutput=False)
    out = nc.declare_dram_parameter("out", [c_loc, b_loc], F32, isOutput=True)

    with tile.TileContext(nc) as tc, ExitStack() as ctx:
        xp = ctx.enter_context(tc.tile_pool(name="xp", bufs=6))
        wp = ctx.enter_context(tc.tile_pool(name="wp", bufs=6))
        hp = ctx.enter_context(tc.tile_pool(name="hp", bufs=2 * len(hch)))
        sp = ctx.enter_context(tc.tile_pool(name="sp", bufs=24))
        op = ctx.enter_context(tc.tile_pool(name="op", bufs=4))
        pp = ctx.enter_context(
            tc.tile_pool(name="pp", bufs=8, space=bass.MemorySpace.PSUM)
        )

        for c in range(c_loc):
            b1t = [sp.tile([128, 1], F32, tag="b1t", name=f"b1t{c}_{i}")
                   for i in range(len(hch))]
            w2t = [sp.tile([128, MID], dt2, tag="w2t", name=f"w2t{c}_{i}")
                   for i in range(len(hch))]
            for i, (h0, hs) in enumerate(hch):
                nc.sync.dma_start(b1t[i][:hs, :], b1[c, h0:h0 + hs, :])
                nc.sync.dma_start(w2t[i][:hs, :], w2[c, h0:h0 + hs, :])
            w3t = sp.tile([MID, 1], dt2, tag="w3t", name=f"w3t{c}")
            b2t = sp.tile([MID, 1], F32, tag="b2t", name=f"b2t{c}")
            b3t = sp.tile([1, 1], F32, tag="b3t", name=f"b3t{c}")
            nc.sync.dma_start(w3t[:, :], w3[c])
            nc.sync.dma_start(b2t[:, :], b2[c])
            nc.sync.dma_start(b3t[:, :], b3[c])

            # stage 1: h1T[h, b] = relu(W1[c].T @ xT[c] + b1[c])
            ps = [[pp.tile([128, 512], F32, tag="ps", name=f"ps{c}_{i}_{j}")
                   for j in range(nj)] for i in range(len(hch))]
            for k in range(nk):
                xtt = xp.tile([128, b_loc], io_dt1, tag="xtt", name=f"xtt{c}_{k}")
                w1t = wp.tile([128, H], io_dt1, tag="w1t", name=f"w1t{c}_{k}")
                nc.sync.dma_start(xtt[:, :], xt[c, k * 128:(k + 1) * 128, :])
                nc.sync.dma_start(w1t[:, :], w1[c, k * 128:(k + 1) * 128, :])
                if round_fp32r:
                    xttr = xp.tile([128, b_loc], dt1, tag="xttr",
                                   name=f"xttr{c}_{k}")
                    w1tr = wp.tile([128, H], dt1, tag="w1tr",
                                   name=f"w1tr{c}_{k}")
                    nc.vector.tensor_copy(xttr[:, :], xtt[:, :])
                    nc.vector.tensor_copy(w1tr[:, :], w1t[:, :])
                    xtt, w1t = xttr, w1tr
                for i, (h0, hs) in enumerate(hch):
                    for j in range(nj):
                        nc.tensor.matmul(
                            ps[i][j][:hs, :],
                            w1t[:, h0:h0 + hs],
                            xtt[:, j * 512:(j + 1) * 512],
                            start=(k == 0),
                            stop=(k == nk - 1),
                        )
            h1t = [hp.tile([128, b_loc], dt2, tag="h1t", name=f"h1t{c}_{i}")
                   for i in range(len(hch))]
            for i, (h0, hs) in enumerate(hch):
                for j in range(nj):
                    nc.scalar.activation(
                        h1t[i][:hs, j * 512:(j + 1) * 512],
                        ps[i][j][:hs, :],
                        RELU,
                        bias=b1t[i][:hs, :],
                    )

            # stage 2: h2T[k, b] = relu(W2[c].T @ h1T + b2[c])
            p2 = [pp.tile([MID, 512], F32, tag="ps", name=f"p2{c}_{j}")
                  for j in range(nj)]
            for i, (h0, hs) in enumerate(hch):
                for j in range(nj):
                    nc.tensor.matmul(
                        p2[j][:, :],
                        w2t[i][:hs, :],
                        h1t[i][:hs, j * 512:(j + 1) * 512],
                        start=(i == 0),
                        stop=(i == len(hch) - 1),
                    )
            h2t = op.tile([MID, b_loc], dt2, tag="h2t", name=f"h2t{c}")
            for j in range(nj):
                nc.scalar.activation(
                    h2t[:, j * 512:(j + 1) * 512], p2[j][:, :], RELU,
                    bias=b2t[:, :],
                )

            # stage 3: scalarsT[c, b] = W3[c].T @ h2T + b3[c]
            p3 = [pp.tile([1, 512], F32, tag="ps", name=f"p3{c}_{j}")
                  for j in range(nj)]
            sct = op.tile([1, b_loc], F32, tag="sct", name=f"sct{c}")
            for j in range(nj):
                nc.tensor.matmul(
                    p3[j][:, :], w3t[:, :], h2t[:, j * 512:(j + 1) * 512],
                    start=True, stop=True,
                )
                nc.scalar.activation(
                    sct[:, j * 512:(j + 1) * 512], p3[j][:, :], IDENT,
                    bias=b3t[:, :],
                )
            nc.sync.dma_start(out[c:c + 1, :], sct[0:1, :])

    nc.compile()
    return nc


_NC_CACHE = {}


def _get_nc():
    key = (B_LOC, C_LOC, N, DT1)
    if key not in _NC_CACHE:
        _NC_CACHE[key] = build_nc()
    return _NC_CACHE[key]


def _to_dt1(arr):
    """Cast a float32 ndarray to DT1's numpy representation."""
    if DT1 == mybir.dt.bfloat16:
        import ml_dtypes
        try:
            import torch
            t = torch.from_numpy(np.ascontiguousarray(arr))
            return t.to(torch.bfloat16).view(torch.uint16).numpy().view(
                ml_dtypes.bfloat16)
        except ImportError:
            return arr.astype(ml_dtypes.bfloat16)
    if DT1 == mybir.dt.float16:
        return np.ascontiguousarray(arr, dtype=np.float16)
    return np.ascontiguousarray(arr, dtype=np.float32)


def _to_dt2(arr):
    if DT1 in (mybir.dt.float16, mybir.dt.bfloat16):
        return _to_dt1(arr)
    return np.ascontiguousarray(arr, dtype=np.float32)


def _transpose_shard(xs):
    """[b_loc, c_loc, n] -> contiguous [c_loc, n, b_loc]."""
    try:
        import torch
        try:
            torch.set_num_threads(max(os.cpu_count() or 1, 1))
        except Exception:
            pass
        return torch.from_numpy(np.ascontiguousarray(xs)).permute(
            1, 2, 0).contiguous().numpy()
    except ImportError:
        return np.ascontiguousarray(np.transpose(xs, (1, 2, 0)))


def kernel(x, W1, b1, W2, b2, W3, b3, Wf1, bf1, Wf2, bf2):
    x = np.asarray(x, dtype=np.float32)
    W1 = np.asarray(W1, dtype=np.float32)
    b1 = np.asarray(b1, dtype=np.float32)
    W2 = np.asarray(W2, dtype=np.float32)
    b2 = np.asarray(b2, dtype=np.float32)
    W3 = np.asarray(W3, dtype=np.float32)
    b3 = np.asarray(b3, dtype=np.float32)

    nc = _get_nc()

    if DT1 == mybir.dt.float16:
        # cast before transposing so the shuffle moves half the bytes
        x = _to_dt1(x)

    in_maps = []
    for ib in range(BG):
        bs = slice(ib * B_LOC, (ib + 1) * B_LOC)
        for ic in range(CG):
            cs = slice(ic * C_LOC, (ic + 1) * C_LOC)
            in_maps.append({
                "xt": _to_dt1(_transpose_shard(x[bs, cs, :])),
                "w1": _to_dt1(W1[cs]),
                "b1": np.ascontiguousarray(b1[cs])[:, :, None],
                "w2": _to_dt2(W2[cs]),
                "b2": np.ascontiguousarray(b2[cs])[:, :, None],
                "w3": _to_dt2(W3[cs])[:, :, None],
                "b3": np.ascontiguousarray(b3[cs])[:, None, None],
            })

    res = run_bass_kernel_spmd(nc, in_maps, list(range(BG * CG)))
    LAST["exec_time_ns"] = res.exec_time_ns
    LAST["results"] = res

    scalars = np.empty((B, C), np.float32)
    idx = 0
    for ib in range(BG):
        bs = slice(ib * B_LOC, (ib + 1) * B_LOC)
        for ic in range(CG):
            cs = slice(ic * C_LOC, (ic + 1) * C_LOC)
            scalars[bs, cs] = res.results[idx]["out"].T
            idx += 1

    # Final tiny MLP (C -> 30 -> lowdim) on host in fp32.
    h = np.maximum(scalars @ np.asarray(Wf1, np.float32)
                   + np.asarray(bf1, np.float32), 0.0)
    return (h @ np.asarray(Wf2, np.float32)
            + np.asarray(bf2, np.float32)).astype(np.float32)



# revision 8
# speedup vs baseline: 1.0981x; 1.0981x over previous
"""Trainium2 Bass kernel for nn_ChannelizedLinearCompression.

Computation (fp32 reference):
    h1      = relu(einsum('bcn,cnh->bch', x, W1) + b1)   # [B, C, H]
    h2      = relu(einsum('bch,chk->bck', h1, W2) + b2)  # [B, C, 10]
    scalars = einsum('bck,ck->bc', h2, W3) + b3          # [B, C]
    out     = relu(scalars @ Wf1 + bf1) @ Wf2 + bf2      # [B, 16]

Sharding: 2 batch groups x 4 channel groups over 8 cores. Each core gets
x^T[c_loc, N, b_loc] (host-transposed so every big DMA is contiguous) and
computes scalars^T[c_loc, b_loc] on device; the tiny final MLP (0.003% of
the FLOPs) runs on host.

Device per-core dataflow (v2):
  The H=286 output rows split into h-chunks [128, 128, 30]. A naive
  M-major loop wastes 26% of PE columns on the 30-row tail. Instead the
  k-sweep interleaves all 3 local channels and col-tiles the three
  30-row tails into ONE psum bank via tile_position=(0, 32c), so they
  stream concurrently on different PE column groups.

  PSUM budget (8 banks): 6 full-M banks (3c x 2 h-chunks) + 1 trio bank
  + 1 stage-2/3 bank. That forces the batch dim into two j-passes of
  F=512. W1 is loaded once into SBUF (just-in-time groups of 4 k-chunks
  during pass j0, host-shuffled so rows are 2.2KB) and reused in j1.
  x tiles stream per (j, k, c) on both HWDGE queues (sync + scalar).
"""

import os
from contextlib import ExitStack

import numpy as np

import concourse.bass as bass
import concourse.tile as tile
from concourse import bacc, mybir
from concourse.bass_utils import run_bass_kernel_spmd
from concourse._compat import get_trn_type

# Problem shapes (hardcoded; kernel.py must be self-contained).
B, C, N = 2048, 12, 8192
H, MID = 286, 10
FINAL_HIDDEN, LOWDIM = 30, 16
BG, CG = 2, 4  # batch groups x channel groups = 8 cores
B_LOC, C_LOC = B // BG, C // CG

NK = N // 128          # 64 contraction chunks
KG = 4                 # W1 k-chunks per DMA group (2.2KB rows)
NKG = NK // KG         # 16 W1 DMA groups per channel
F = 512                # j-pass width (one PSUM bank of fp32)
NJ = B_LOC // F        # 2 j-passes
HFULL = [(0, 128), (128, 128)]  # full-M h-chunks
H3_0, H3_S = 256, 30            # the 30-row tail chunk

F16 = mybir.dt.float16
F32 = mybir.dt.float32
RELU = mybir.ActivationFunctionType.Relu
IDENT = mybir.ActivationFunctionType.Identity

LAST = {}  # introspection for test.py (exec_time_ns etc.); harness ignores


def build_nc():
    nc = bacc.Bacc(get_trn_type() or "TRN2", target_bir_lowering=False)
    xt = nc.declare_dram_parameter("xt", [C_LOC, N, B_LOC], F16, isOutput=False)
    w1 = nc.declare_dram_parameter("w1", [C_LOC, NKG, 128, KG * H], F16,
                                   isOutput=False)
    b1 = nc.declare_dram_parameter("b1", [C_LOC, H, 1], F32, isOutput=False)
    w2 = nc.declare_dram_parameter("w2", [C_LOC, H, MID], F16, isOutput=False)
    b2 = nc.declare_dram_parameter("b2", [C_LOC, MID, 1], F32, isOutput=False)
    w3 = nc.declare_dram_parameter("w3", [C_LOC, MID, 1], F16, isOutput=False)
    b3 = nc.declare_dram_parameter("b3", [C_LOC, 1, 1], F32, isOutput=False)
    out = nc.declare_dram_parameter("out", [C_LOC, B_LOC], F32, isOutput=True)

    with tile.TileContext(nc) as tc, ExitStack() as ctx:
        xp = ctx.enter_context(tc.tile_pool(name="xp", bufs=24))
        wp = ctx.enter_context(tc.tile_pool(name="wp", bufs=1))
        hp = ctx.enter_context(tc.tile_pool(name="hp", bufs=2))
        sp = ctx.enter_context(tc.tile_pool(name="sp", bufs=1))
        op = ctx.enter_context(tc.tile_pool(name="op", bufs=2))
        pp = ctx.enter_context(
            tc.tile_pool(name="pp", bufs=1, space=bass.MemorySpace.PSUM)
        )

        dmae = [nc.sync, nc.scalar]  # the two HWDGE queues

        # W1 resident tiles, one per channel, filled JIT during pass j0.
        w1r = [wp.tile([128, NK * H], F16, tag=f"w1r{c}", name=f"w1r{c}")
               for c in range(C_LOC)]

        # Kick off the first x tiles and first W1 groups before anything
        # else so the PE can start ASAP.
        xtt = {}
        PRE = 6  # x prefetch depth in (j, k) steps

        def fetch_x(s):
            if s >= NJ * NK:
                return
            j, k = divmod(s, NK)
            for c in range(C_LOC):
                t = xp.tile([128, F], F16, tag="xtt", name=f"xtt{j}_{k}_{c}")
                dmae[(s * C_LOC + c) % 2].dma_start(
                    t[:, :], xt[c, k * 128:(k + 1) * 128, j * F:(j + 1) * F])
                xtt[(j, k, c)] = t

        for s in range(2):
            fetch_x(s)
        for c in range(C_LOC):
            dmae[c % 2].dma_start(w1r[c][:, 0:KG * H], w1[c, 0])
        for s in range(2, PRE):
            fetch_x(s)

        # Small per-channel tensors (biases, W2, W3).
        b1t = [[sp.tile([128, 1], F32, tag=f"b1t{c}_{i}", name=f"b1t{c}_{i}")
                for i in range(2)] for c in range(C_LOC)]
        b1t3 = sp.tile([96, 1], F32, tag="b1t3", name="b1t3")
        w2t = [[sp.tile([128, MID], F16, tag=f"w2t{c}_{i}", name=f"w2t{c}_{i}")
                for i in range(2)] for c in range(C_LOC)]
        w2t3 = sp.tile([96, MID], F16, tag="w2t3", name="w2t3")
        w3t = [sp.tile([MID, 1], F16, tag=f"w3t{c}", name=f"w3t{c}")
               for c in range(C_LOC)]
        b2t = [sp.tile([MID, 1], F32, tag=f"b2t{c}", name=f"b2t{c}")
               for c in range(C_LOC)]
        b3t = [sp.tile([1, 1], F32, tag=f"b3t{c}", name=f"b3t{c}")
               for c in range(C_LOC)]
        for c in range(C_LOC):
            for i, (h0, hs) in enumerate(HFULL):
                nc.scalar.dma_start(b1t[c][i][:hs, :], b1[c, h0:h0 + hs, :])
                nc.scalar.dma_start(w2t[c][i][:hs, :], w2[c, h0:h0 + hs, :])
            nc.scalar.dma_start(b1t3[32 * c:32 * c + H3_S, :],
                                b1[c, H3_0:H3_0 + H3_S, :])
            nc.scalar.dma_start(w2t3[32 * c:32 * c + H3_S, :],
                                w2[c, H3_0:H3_0 + H3_S, :])
            nc.scalar.dma_start(w3t[c][:, :], w3[c])
            nc.scalar.dma_start(b2t[c][:, :], b2[c])
            nc.scalar.dma_start(b3t[c][:, :], b3[c])

        sct = [op.tile([1, B_LOC], F32, tag=f"sct{c}", name=f"sct{c}")
               for c in range(C_LOC)]

        for j in range(NJ):
            # Stage 1: psum accumulation over the k sweep, channels
            # interleaved; 30-row tails col-tiled into one trio bank.
            ps = [[pp.tile([128, F], F32, tag=f"ps{c}_{i}",
                           name=f"ps{j}_{c}_{i}")
                   for i in range(2)] for c in range(C_LOC)]
            ps3 = pp.tile([96, F], F32, tag="ps3", name=f"ps3_{j}")
            for k in range(NK):
                # prefetch: x tiles PRE steps ahead, W1 groups in pass j0
                if j == 0 and k % KG == 0 and k + KG < NK:
                    kk = k // KG + 1
                    for c in range(C_LOC):
                        dmae[(kk + c) % 2].dma_start(
                            w1r[c][:, kk * KG * H:(kk + 1) * KG * H],
                            w1[c, kk])
                fetch_x(j * NK + k + PRE)
                start, stop = k == 0, k == NK - 1
                for c in range(C_LOC):
                    xk = xtt[(j, k, c)]
                    for i, (h0, hs) in enumerate(HFULL):
                        nc.tensor.matmul(
                            ps[c][i][:, :],
                            w1r[c][:, k * H + h0:k * H + h0 + hs],
                            xk[:, :],
                            start=start, stop=stop,
                        )
                for c in range(C_LOC):
                    nc.tensor.matmul(
                        ps3[32 * c:32 * c + H3_S, :],
                        w1r[c][:, k * H + H3_0:k * H + H3_0 + H3_S],
                        xtt[(j, k, c)][:, :],
                        start=start, stop=stop,
                        tile_position=(0, 32 * c),
                    )
                for c in range(C_LOC):
                    del xtt[(j, k, c)]

            # Evict h1 (relu + bias) in the same order the banks stopped.
            h1t = [[hp.tile([128, F], F16, tag=f"h1t{c}_{i}",
                            name=f"h1t{j}_{c}_{i}") for i in range(2)]
                   for c in range(C_LOC)]
            h1t3 = hp.tile([96, F], F16, tag="h1t3", name=f"h1t3_{j}")
            for c in range(C_LOC):
                for i, (h0, hs) in enumerate(HFULL):
                    nc.scalar.activation(h1t[c][i][:hs, :], ps[c][i][:hs, :],
                                         RELU, bias=b1t[c][i][:hs, :])
            for c in range(C_LOC):
                nc.scalar.activation(
                    h1t3[32 * c:32 * c + H3_S, :],
                    ps3[32 * c:32 * c + H3_S, :],
                    RELU, bias=b1t3[32 * c:32 * c + H3_S, :])

            # Stage 2 + 3 per channel for this j-half.
            for c in range(C_LOC):
                p2 = pp.tile([MID, F], F32, tag="p23", name=f"p2_{j}_{c}")
                for i, (h0, hs) in enumerate(HFULL):
                    nc.tensor.matmul(p2[:, :], w2t[c][i][:hs, :],
                                     h1t[c][i][:hs, :],
                                     start=(i == 0), stop=False)
                nc.tensor.matmul(p2[:, :], w2t3[32 * c:32 * c + H3_S, :],
                                 h1t3[32 * c:32 * c + H3_S, :],
                                 start=False, stop=True)
                h2t = op.tile([MID, F], F16, tag="h2t", name=f"h2t{j}_{c}")
                nc.scalar.activation(h2t[:, :], p2[:, :], RELU,
                                     bias=b2t[c][:, :])
                p3 = pp.tile([1, F], F32, tag="p23", name=f"p3_{j}_{c}")
                nc.tensor.matmul(p3[:, :], w3t[c][:, :], h2t[:, :],
                                 start=True, stop=True)
                nc.scalar.activation(sct[c][0:1, j * F:(j + 1) * F],
                                     p3[:, :], IDENT, bias=b3t[c][:, :])
                nc.sync.dma_start(out[c:c + 1, j * F:(j + 1) * F],
                                  sct[c][0:1, j * F:(j + 1) * F])

    nc.compile()
    return nc


_NC_CACHE = {}


def _get_nc():
    if "nc" not in _NC_CACHE:
        _NC_CACHE["nc"] = build_nc()
    return _NC_CACHE["nc"]


def _f16(arr):
    return np.ascontiguousarray(arr, dtype=np.float16)


def _transpose_shard(xs):
    """[b_loc, c_loc, n] -> contiguous [c_loc, n, b_loc]."""
    try:
        import torch
        try:
            torch.set_num_threads(max(os.cpu_count() or 1, 1))
        except Exception:
            pass
        return torch.from_numpy(np.ascontiguousarray(xs)).permute(
            1, 2, 0).contiguous().numpy()
    except ImportError:
        return np.ascontiguousarray(np.transpose(xs, (1, 2, 0)))


def _shuffle_w1(w1c):
    """[c_loc, N, H] -> [c_loc, NKG, 128, KG*H] with
    out[c, kk, p, i*H:(i+1)*H] = w1c[c, (kk*KG+i)*128 + p, :]."""
    r = w1c.reshape(C_LOC, NKG, KG, 128, H)
    return np.ascontiguousarray(r.transpose(0, 1, 3, 2, 4)).reshape(
        C_LOC, NKG, 128, KG * H)


def kernel(x, W1, b1, W2, b2, W3, b3, Wf1, bf1, Wf2, bf2):
    x = np.asarray(x, dtype=np.float32)
    W1 = np.asarray(W1, dtype=np.float32)
    b1 = np.asarray(b1, dtype=np.float32)
    W2 = np.asarray(W2, dtype=np.float32)
    b2 = np.asarray(b2, dtype=np.float32)
    W3 = np.asarray(W3, dtype=np.float32)
    b3 = np.asarray(b3, dtype=np.float32)

    nc = _get_nc()

    # cast before transposing so the shuffle moves half the bytes
    x = _f16(x)

    in_maps = []
    for ib in range(BG):
        bs = slice(ib * B_LOC, (ib + 1) * B_LOC)
        for ic in range(CG):
            cs = slice(ic * C_LOC, (ic + 1) * C_LOC)
            in_maps.append({
                "xt": _f16(_transpose_shard(x[bs, cs, :])),
                "w1": _shuffle_w1(_f16(W1[cs])),
                "b1": np.ascontiguousarray(b1[cs])[:, :, None],
                "w2": _f16(W2[cs]),
                "b2": np.ascontiguousarray(b2[cs])[:, :, None],
                "w3": _f16(W3[cs])[:, :, None],
                "b3": np.ascontiguousarray(b3[cs])[:, None, None],
            })

    res = run_bass_kernel_spmd(nc, in_maps, list(range(BG * CG)))
    LAST["exec_time_ns"] = res.exec_time_ns
    LAST["results"] = res

    scalars = np.empty((B, C), np.float32)
    idx = 0
    for ib in range(BG):
        bs = slice(ib * B_LOC, (ib + 1) * B_LOC)
        for ic in range(CG):
            cs = slice(ic * C_LOC, (ic + 1) * C_LOC)
            scalars[bs, cs] = res.results[idx]["out"].T
            idx += 1

    # Final tiny MLP (C -> 30 -> lowdim) on host in fp32.
    h = np.maximum(scalars @ np.asarray(Wf1, np.float32)
                   + np.asarray(bf1, np.float32), 0.0)
    return (h @ np.asarray(Wf2, np.float32)
            + np.asarray(bf2, np.float32)).astype(np.float32)
